# revision 21
# baseline (speedup 1.0000x reference)
# MoE-ViT forward pass on 8 trn2 NeuronCores.
# Attention is data-parallel (1 image/core); the MoE MLP is pair
# expert-parallel: cores 0-3 hold experts {0,1}, cores 4-7 hold {2,3}, and
# pair (c, c+4) swaps LN2 activations (pair AllGather), each computing its
# two experts dense-masked over both images, then pair-ReduceScatter(add)
# returns each image's combined MoE output to its home core.  Gate columns
# are host-permuted per core so "my experts" are always columns 0..1;
# routing math stays fp32 and bit-identical to the data-parallel version.
# Layout: activations feature-major [D on partitions (6x128), tokens on free].
# Matmuls in bf16 (fp32 PSUM accumulation), LN/softmax/routing math in fp32.
# Weights are identical on every core, so each core receives only a 1/8
# shard over the (slow) host link; full tensors are reassembled on device
# with AllGather collectives (4-core groups for the expert halves) that are
# posted two layers ahead of use, interleaved with the pair exchanges since
# collectives execute strictly in posted order.
import os
import numpy as np
import ml_dtypes

import concourse.bass as bass
import concourse.bacc as bacc
import concourse.mybir as mybir
import concourse.tile as tile
from concourse.tile import ScopedClock
from concourse.bass_utils import run_bass_kernel_spmd
from concourse.masks import make_identity
import bass_rust

F32 = mybir.dt.float32
BF16 = mybir.dt.bfloat16
AF = mybir.ActivationFunctionType
ALU = mybir.AluOpType

B, IMG, PATCH, CIN = 8, 224, 16, 3
D, NH, E, HID, NDEPTH, NC = 768, 12, 4, 3072, 12, 1000
NPAT = (IMG // PATCH) ** 2  # 196
S = NPAT + 1  # 197
HD = D // NH  # 64
EPS = 1e-5
KC = D // 128  # 6 feature chunks
HC = HID // 128  # 24 hidden chunks
HCH = HC // 2  # 12 hidden chunks per half
N_CORES = 8
DEPTH = int(os.environ.get("VIT_DEPTH", NDEPTH))
DEBUG_H = bool(int(os.environ.get("VIT_DEBUG_H", "0")))
GATHER = bool(int(os.environ.get("VIT_GATHER", "1")))
COMPUTE = bool(int(os.environ.get("VIT_COMPUTE", "1")))
# pair expert-parallel MoE: cores 0-3 own experts {0,1}, cores 4-7 own
# {2,3}; pair (c, c+4) exchanges activations and splits the expert work
# for its two images.  Requires GATHER.
EP = bool(int(os.environ.get("VIT_EP", "0"))) and GATHER
EH = E // 2  # experts resident per core under EP
S2 = 2 * S  # tokens of the two pair images, concatenated
GFULL = [list(range(N_CORES))]
G4 = [[0, 1, 2, 3], [4, 5, 6, 7]]  # expert-half gather groups
GPAIR = [[0, 4], [1, 5], [2, 6], [3, 7]]  # image-pair exchange groups

_LAST_DECLS = {}
SCHUNKS = [(0, 128), (128, S - 128)]  # token chunks (start, width)
NCHUNKS = [(0, 512), (512, 256)]  # dout free-dim chunks for token-major mm

# elements per core-shard of each gathered tensor ( / layer for per-layer)
SH_QKV = 128 * KC * 3 * D // N_CORES  # 221184
SH_WO = 128 * KC * D // N_CORES  # 73728
SH_W1 = 128 * E * KC * HID // N_CORES  # 1179648
SH_W2 = 128 * E * HC * D // N_CORES  # 1179648
SH_PW = 128 * KC * D // N_CORES
SH_POS = 128 * KC * S // N_CORES
SH_HW = 128 * KC * NC // N_CORES


def _patched_drain(self, tick_clock, wait_clock):
    # Upstream _drain_and_barrier puts every outstanding proc's sem wait on
    # the single Drain instruction; walrus CoreV3 codegen caps inline sync
    # waits below that. Split: one carrier nop per proc, each with one wait.
    gc_vals = eval(repr(tick_clock.global_clock)[len("VectorClock("):-1])
    for proc, _handle in self.sems.allocated().items():
        single = bass_rust.VectorClock(
            [v if i == proc else 0 for i, v in enumerate(gc_vals)])
        nop = self.nc.sync.nop(nofuse=True)
        wait_clock.add_sem_waits(nop.ins, ScopedClock({None: single}))
    self.nc.sync.drain()
    self.nc.all_engine_barrier()
    popped = self.nc._tile_sem_poison_stack.pop()
    assert popped is self._sem_poison
    self.nc.clear_and_free_semaphores(list(self.sems.allocated().values()))
    self.nc.all_engine_barrier()


tile.TileContext._drain_and_barrier = _patched_drain


def _build(depth):
    nc = bacc.Bacc("TRN2", target_bir_lowering=False, debug=False,
                   num_devices=N_CORES)

    def din(name, shape, dt=F32):
        return nc.dram_tensor(name, shape, dt, kind="ExternalInput").ap()

    t = {}
    t["xp"] = din("xp", [128, KC, NPAT], BF16)
    t["patch_b"] = din("patch_b", [128, KC])
    t["bqkv"] = din("bqkv", [128, NDEPTH, 18])
    t["bqkv_row"] = din("bqkv_row", [1, NDEPTH, 3 * D], BF16)
    t["bo"] = din("bo", [128, NDEPTH, KC])
    t["ln1g"] = din("ln1g", [128, NDEPTH, KC])
    t["ln1b"] = din("ln1b", [128, NDEPTH, KC])
    t["ln2g"] = din("ln2g", [128, NDEPTH, KC])
    t["ln2b"] = din("ln2b", [128, NDEPTH, KC])
    t["gwT32"] = din("gwT32", [128, NDEPTH, KC, E])
    t["gb_row"] = din("gb_row", [1, NDEPTH, E])
    EB = EH if EP else E
    t["b1b"] = din("b1b", [128, NDEPTH, EB, HC])
    t["b2row"] = din("b2row", [1, NDEPTH, EB, D], BF16)
    t["head_b"] = din("head_b", [1, NC])
    if GATHER and not EP:
        # per-core 1/8 shards, layer-PAIR-major: one AllGather reassembles
        # two layers' weights, halving collective count
        t["wqkvT_s"] = din("wqkvT_s", [NDEPTH // 2, 2 * SH_QKV], BF16)
        t["woT_s"] = din("woT_s", [NDEPTH // 2, 2 * SH_WO], BF16)
        t["w1T_s"] = din("w1T_s", [NDEPTH // 2, 2 * SH_W1], BF16)
        t["w2T_s"] = din("w2T_s", [NDEPTH // 2, 2 * SH_W2], BF16)
        t["pwT_s"] = din("pwT_s", [SH_PW], BF16)
        t["posT_s"] = din("posT_s", [SH_POS])
        t["hwT_s"] = din("hwT_s", [SH_HW], BF16)
    elif GATHER:
        # per-core 1/8 shards of the replicated weights (layer-major)
        t["wqkvT_s"] = din("wqkvT_s", [NDEPTH, SH_QKV], BF16)
        t["woT_s"] = din("woT_s", [NDEPTH, SH_WO], BF16)
        t["w1T_s"] = din("w1T_s", [NDEPTH, SH_W1], BF16)
        t["w2T_s"] = din("w2T_s", [NDEPTH, SH_W2], BF16)
        t["pwT_s"] = din("pwT_s", [SH_PW], BF16)
        t["posT_s"] = din("posT_s", [SH_POS])
        t["hwT_s"] = din("hwT_s", [SH_HW], BF16)
    else:
        t["wqkvT_g"] = din("wqkvT_g", [NDEPTH, 128, KC, 3 * D], BF16)
        t["woT_g"] = din("woT_g", [NDEPTH, 128, KC, D], BF16)
        t["w1T_g"] = din("w1T_g", [NDEPTH, 128, E, KC, HID], BF16)
        t["w2T_g"] = din("w2T_g", [NDEPTH, 128, E, HC, D], BF16)
        t["pwT_g"] = din("pwT_g", [128, KC, D], BF16)
        t["posT_g"] = din("posT_g", [128, KC, S])
        t["hwT_g"] = din("hwT_g", [128, KC, NC], BF16)

    t["out"] = nc.dram_tensor("out", [1, NC], F32, kind="ExternalOutput").ap()
    _LAST_DECLS.clear()
    _LAST_DECLS.update({k: v for k, v in t.items() if v is not None and k != "out"})
    t["hdbg"] = None
    if DEBUG_H:
        t["hdbg"] = nc.dram_tensor("hdbg", [128, KC, S], F32,
                                   kind="ExternalOutput").ap()

    with tile.TileContext(nc) as tc:
        _emit(nc, tc, depth, t)
    nc.finalize()
    return nc


def _emit_gathers(nc, tc, ctx, depth, t):
    """Weight distribution via AllGather from per-core shards.

    Collectives execute strictly in posted order, so the big weight
    gathers are NOT all posted up front: layers 0/1 (plus the small
    shared tensors) are posted at startup and the layer loop posts layer
    l+2's gathers mid-layer, keeping the CC queue drained ahead of the
    latency-critical pair exchanges (EP mode).  Returns (g, postA, postB)
    where postA(l)/postB(l) post the two halves of layer l's gathers."""
    if not GATHER:
        return ({"qkv": [t["wqkvT_g"][l] for l in range(NDEPTH)],
                 "wo": [t["woT_g"][l] for l in range(NDEPTH)],
                 "w1": [t["w1T_g"][l] for l in range(NDEPTH)],
                 "w2": [t["w2T_g"][l] for l in range(NDEPTH)],
                 "pw": t["pwT_g"], "pos": t["posT_g"], "hw": t["hwT_g"]},
                lambda l: None, lambda l: None)
    dram = ctx.enter_context(tc.tile_pool(name="gdram", bufs=1, space="DRAM"))

    def gather(name, src_ap, shard_el, full_shape, dt, groups):
        bounce = dram.tile([shard_el], dt, tag=f"b_{name}", name=f"b_{name}")
        nc.sync.dma_start(bounce[:], src_ap)
        shared = "Shared" if len(groups[0]) > 4 else "Local"
        out = dram.tile(list(full_shape), dt, addr_space=shared,
                        tag=f"g_{name}", name=f"g_{name}")
        nc.gpsimd.collective_compute(
            "AllGather", ALU.bypass, replica_groups=groups,
            ins=[bounce[:].opt()], outs=[out[:].opt()])
        return out

    g = {"qkv": [None] * NDEPTH, "wo": [None] * NDEPTH,
         "w1": [None] * NDEPTH, "w2": [None] * NDEPTH}

    def postA(l):
        if l >= depth or g["qkv"][l] is not None:
            return
        if EP:
            g["qkv"][l] = gather(f"qkv{l}", t["wqkvT_s"][l], SH_QKV,
                                 [128, KC, 3 * D], BF16, GFULL)
            g["w1"][l] = gather(f"w1{l}", t["w1T_s"][l], SH_W1,
                                [EH, 128, KC, HID], BF16, G4)
            return
        p, lw = l // 2, min(2, depth - (l // 2) * 2)
        gq = gather(f"qkv{p}", t["wqkvT_s"][p][:lw * SH_QKV],
                    lw * SH_QKV, [lw, 128, KC, 3 * D], BF16, GFULL)
        gw1 = gather(f"w1{p}", t["w1T_s"][p][:lw * SH_W1],
                     lw * SH_W1, [lw, 128, E, KC, HID], BF16, GFULL)
        for j in range(lw):
            g["qkv"][2 * p + j] = gq[j]
            g["w1"][2 * p + j] = gw1[j]

    def postB(l):
        if l >= depth or g["wo"][l] is not None:
            return
        if EP:
            g["wo"][l] = gather(f"wo{l}", t["woT_s"][l], SH_WO,
                                [128, KC, D], BF16, GFULL)
            g["w2"][l] = gather(f"w2{l}", t["w2T_s"][l], SH_W2,
                                [EH, 128, HC, D], BF16, G4)
            return
        p, lw = l // 2, min(2, depth - (l // 2) * 2)
        gwo = gather(f"wo{p}", t["woT_s"][p][:lw * SH_WO],
                     lw * SH_WO, [lw, 128, KC, D], BF16, GFULL)
        gw2 = gather(f"w2{p}", t["w2T_s"][p][:lw * SH_W2],
                     lw * SH_W2, [lw, 128, E, HC, D], BF16, GFULL)
        for j in range(lw):
            g["wo"][2 * p + j] = gwo[j]
            g["w2"][2 * p + j] = gw2[j]

    g["pw"] = gather("pw", t["pwT_s"][:], SH_PW, [128, KC, D], BF16, GFULL)
    g["pos"] = gather("pos", t["posT_s"][:], SH_POS, [128, KC, S], F32,
                      GFULL)
    for l in range(min(2, depth)):
        postA(l)
        postB(l)
    g["hw"] = gather("hw", t["hwT_s"][:], SH_HW, [128, KC, NC], BF16, GFULL)
    return g, postA, postB


def _emit(nc, tc, depth, t):
    import contextlib
    ctx = contextlib.ExitStack()
    with ctx:
        def _env(name, dflt):
            return int(os.environ.get(name, dflt))
        g, postA, postB = _emit_gathers(nc, tc, ctx, depth, t)
        if GATHER and COMPUTE and not EP:
            for l in range(2, depth, 2):
                postA(l)
                postB(l)
        if EP:
            dram2 = ctx.enter_context(
                tc.tile_pool(name="xdram", bufs=2, space="DRAM"))
        if not COMPUTE:
            # gathers-only benchmark mode: touch one gathered byte per tensor
            # so nothing is dead, write a dummy output
            for l in range(2, depth):
                postA(l)
                postB(l)
            sbp = ctx.enter_context(tc.tile_pool(name="sbp", bufs=1))
            acc = sbp.tile([1, NC], F32)
            nc.vector.memset(acc[:], 0.0)
            probes = [g["pw"], g["pos"], g["hw"]] + \
                [g[k][l] for k in ("qkv", "wo", "w1", "w2")
                 for l in range(depth)]
            for i, ap in enumerate(probes):
                pr = sbp.tile([1, 4], F32 if ap is g["pos"] else BF16,
                              tag="probe", bufs=2, name=f"probe{i}")
                if len(ap.shape) == 3:
                    src = ap[:1, 0, 0:4]
                elif EP:
                    src = ap[0, :1, 0, 0:4]
                else:
                    src = ap[:1, 0, 0, 0:4]
                nc.sync.dma_start(pr[:], src)
                pf = sbp.tile([1, 4], F32, tag="probef", bufs=2,
                              name=f"probef{i}")
                nc.vector.tensor_copy(pf[:], pr[:])
                nc.vector.tensor_tensor(acc[:, 0:4], acc[:, 0:4], pf[:],
                                        ALU.add)
            nc.sync.dma_start(t["out"][:], acc[:])
            return
        state = ctx.enter_context(tc.tile_pool(name="state", bufs=1))
        wpool = ctx.enter_context(
            tc.tile_pool(name="wpool", bufs=_env("VIT_WPOOL", 1)))
        wmoe = ctx.enter_context(
            tc.tile_pool(name="wmoe", bufs=_env("VIT_WMOE", 2)))
        act = ctx.enter_context(tc.tile_pool(name="act", bufs=1))
        act2 = ctx.enter_context(
            tc.tile_pool(name="act2", bufs=_env("VIT_ACT2", 2)))
        mm = ctx.enter_context(
            tc.tile_pool(name="mm", bufs=_env("VIT_MM", 6), space="PSUM"))
        stat = ctx.enter_context(tc.tile_pool(name="stat", bufs=1,
                                              space="PSUM"))
        tp = ctx.enter_context(
            tc.tile_pool(name="tp", bufs=_env("VIT_TP", 2), space="PSUM"))

        # ---- persistent small tiles ----
        ones_col_f32 = state.tile([128, 1], F32)  # lhsT for partition sums
        nc.vector.memset(ones_col_f32[:], 1.0)
        ones_row_f32 = state.tile([1, 128], F32)  # lhsT for bcast (K=1)
        nc.vector.memset(ones_row_f32[:], 1.0)
        ones_row_bf = state.tile([1, 128], BF16)
        nc.vector.memset(ones_row_bf[:], 1.0)
        ident_bf = state.tile([128, 128], BF16)
        make_identity(nc, ident_bf[:])
        ident_f32 = state.tile([128, 128], F32)
        make_identity(nc, ident_f32[:])
        eps_t = state.tile([1, 1], F32)
        nc.vector.memset(eps_t[:], EPS)
        zero_col = state.tile([128, 1], F32)
        nc.vector.memset(zero_col[:], 0.0)

        h = state.tile([128, KC, S], F32)
        nc.sync.dma_start(h[:], g["pos"][:])

        lnp = {}
        for name in ("ln1g", "ln1b", "ln2g", "ln2b", "bqkv", "bo", "patch_b",
                     "b1b"):
            lnp[name] = state.tile(list(t[name].shape), F32, name=name)
            nc.sync.dma_start(lnp[name][:], t[name][:])
        gb_row_sb = state.tile([1, NDEPTH, E], F32)
        nc.sync.dma_start(gb_row_sb[:], t["gb_row"][:])

        # ---- patch embedding: h[:, m, 1:] += pwT.T @ xp + patch_b ----
        xp_sb = wmoe.tile([128, KC, NPAT], BF16, tag="w1q", name="xp_sb")
        nc.sync.dma_start(xp_sb[:], t["xp"][:])
        pw_sb = wmoe.tile([128, KC, D], BF16, tag="w2q", name="pw_sb")
        nc.sync.dma_start(pw_sb[:], g["pw"][:])
        for m in range(KC):
            ps = mm.tile([128, 512], F32, tag="mm")
            for kc in range(KC):
                nc.tensor.matmul(ps[:, :NPAT],
                                 pw_sb[:, kc, 128 * m:128 * (m + 1)],
                                 xp_sb[:, kc, :], start=(kc == 0),
                                 stop=(kc == KC - 1))
            tmp = act2.tile([128, NPAT], F32, tag="embtmp")
            nc.scalar.activation(tmp[:], ps[:, :NPAT], AF.Identity,
                                 bias=lnp["patch_b"][:, m:m + 1], scale=1.0)
            nc.vector.tensor_tensor(h[:, m, 1:S], h[:, m, 1:S], tmp[:],
                                    ALU.add)

        def layernorm(l, gk, bk, out_bf16, out_f32=None):
            # h [128, KC, S] fp32 -> out_bf16 [128, KC, S] = norm(h)*g + b
            hsq = act.tile([128, KC, S], F32, tag="hsq")
            for kc in range(KC):
                nc.vector.tensor_tensor(hsq[:, kc, :], h[:, kc, :],
                                        h[:, kc, :], ALU.mult)
            mean = act.tile([1, S], F32, tag="lnmean")
            em2 = act.tile([1, S], F32, tag="lnem2")
            for dst, src in ((mean, h), (em2, hsq)):
                pstat = mm.tile([128, 512], F32, tag="mm", name="pstat")
                for kc in range(KC):
                    nc.tensor.matmul(pstat[:1, :S], ones_col_f32[:],
                                     src[:, kc, :], start=(kc == 0),
                                     stop=(kc == KC - 1))
                nc.vector.tensor_scalar_mul(dst[:], pstat[:1, :S], 1.0 / D)
            var = act.tile([1, S], F32, tag="lnvar")
            nc.vector.tensor_tensor(var[:], mean[:], mean[:], ALU.mult)
            nc.vector.tensor_tensor(var[:], em2[:], var[:], ALU.subtract)
            sd = act.tile([1, S], F32, tag="lnsd")
            nc.scalar.activation(sd[:], var[:], AF.Sqrt, bias=eps_t[:], scale=1.0)
            rstd = act.tile([1, S], F32, tag="lnrstd")
            nc.vector.reciprocal(rstd[:], sd[:])
            nmr = act.tile([1, S], F32, tag="lnnmr")
            nc.vector.tensor_tensor(nmr[:], mean[:], rstd[:], ALU.mult)
            nc.vector.tensor_scalar_mul(nmr[:], nmr[:], -1.0)
            pb1t = mm.tile([128, 512], F32, tag="mm", name="pb1")
            pb1 = pb1t[:, :S]
            nc.tensor.matmul(pb1, ones_row_f32[:], rstd[:], start=True,
                             stop=True)
            pb2t = mm.tile([128, 512], F32, tag="mm", name="pb2")
            pb2 = pb2t[:, :S]
            nc.tensor.matmul(pb2, ones_row_f32[:], nmr[:], start=True,
                             stop=True)
            for kc in range(KC):
                tmp = act2.tile([128, S], F32, tag="lntmp")
                nc.vector.tensor_tensor(tmp[:], h[:, kc, :], pb1, ALU.mult)
                nc.vector.tensor_tensor(tmp[:], tmp[:], pb2, ALU.add)
                if out_f32 is not None:
                    nc.scalar.activation(out_f32[:, kc, :], tmp[:],
                                         AF.Identity,
                                         bias=lnp[bk][:, l, kc:kc + 1],
                                         scale=lnp[gk][:, l, kc:kc + 1])
                    if out_bf16 is not None:
                        nc.vector.tensor_copy(out_bf16[:, kc, :],
                                              out_f32[:, kc, :])
                else:
                    nc.scalar.activation(out_bf16[:, kc, :], tmp[:],
                                         AF.Identity,
                                         bias=lnp[bk][:, l, kc:kc + 1],
                                         scale=lnp[gk][:, l, kc:kc + 1])

        for l in range(depth):
            wqkv = wpool.tile([128, KC, 3 * D], BF16, tag="wqkv")
            nc.sync.dma_start(wqkv[:], g["qkv"][l][:])
            wo = wpool.tile([128, KC, D], BF16, tag="wo")
            nc.sync.dma_start(wo[:], g["wo"][l][:])
            gw = wpool.tile([128, KC, E], F32, tag="gw")
            nc.sync.dma_start(gw[:], t["gwT32"][:, l])
            bqkv_row_sb = act2.tile([1, 3 * D], BF16, tag="bqkvr")
            nc.sync.dma_start(bqkv_row_sb[:], t["bqkv_row"][:, l])
            b2row_sb = act2.tile([1, EH if EP else E, D], BF16, tag="b2r")
            nc.sync.dma_start(b2row_sb[:], t["b2row"][:, l])

            # ---- attention ----
            qn = act.tile([128, KC, S], BF16, tag="qn")
            layernorm(l, "ln1g", "ln1b", qn)
            hbf = act.tile([128, KC, S], BF16, tag="hbf")
            for kc in range(KC):
                nc.vector.tensor_copy(hbf[:, kc, :], h[:, kc, :])

            qb = act.tile([128, KC, S], BF16, tag="qb")
            kb = act.tile([128, KC, S], BF16, tag="kb")
            for j, (src, dst) in enumerate(((qn, qb), (hbf, kb))):
                for m in range(KC):
                    ps = mm.tile([128, 512], F32, tag="mm")
                    for kc in range(KC):
                        nc.tensor.matmul(
                            ps[:, :S],
                            wqkv[:, kc, j * D + 128 * m:j * D + 128 * (m + 1)],
                            src[:, kc, :], start=(kc == 0),
                            stop=(kc == KC - 1))
                    nc.scalar.activation(
                        dst[:, m, :], ps[:, :S], AF.Identity,
                        bias=lnp["bqkv"][:, l, j * KC + m:j * KC + m + 1],
                        scale=1.0)
            # v token-major: vT [s-chunk partitions, 2, D]
            vT = act.tile([128, 2, D], BF16, tag="vT")
            for si, (s0, sw) in enumerate(SCHUNKS):
                for (n0, nw) in NCHUNKS:
                    ps = mm.tile([128, 512], F32, tag="mm")
                    for kc in range(KC):
                        nc.tensor.matmul(
                            ps[:sw, :nw], hbf[:, kc, s0:s0 + sw],
                            wqkv[:, kc, 2 * D + n0:2 * D + n0 + nw],
                            start=(kc == 0), stop=False)
                    nc.tensor.matmul(
                        ps[:sw, :nw], ones_row_bf[:, :sw],
                        bqkv_row_sb[:, 2 * D + n0:2 * D + n0 + nw],
                        start=False, stop=True)
                    nc.vector.tensor_copy(vT[:sw, si, n0:n0 + nw],
                                          ps[:sw, :nw])

            ob = act.tile([128, KC, S], BF16, tag="ob")
            scale = 1.0 / float(np.sqrt(HD))

            def _score(head):
                hc, hoff = head // 2, (head % 2) * 64
                q_h = qb[hoff:hoff + 64, hc, :]
                k_h = kb[hoff:hoff + 64, hc, :]
                tiles = []
                for qi, (q0, qw) in enumerate(SCHUNKS):
                    ps = mm.tile([128, 512], F32, tag="mm",
                                 name=f"ps_h{head}q{qi}")
                    nc.tensor.matmul(ps[:qw, :S], q_h[:, q0:q0 + qw], k_h,
                                     start=True, stop=True)
                    tiles.append(ps)
                return tiles

            pend = _score(0)
            for head in range(NH):
                hc, hoff = head // 2, (head % 2) * 64
                cur = pend
                if head + 1 < NH:
                    pend = _score(head + 1)
                pav = mm.tile([128, 512], F32, tag="mm")
                attnT = act2.tile([128, 2, S], BF16, tag="attnT")
                for qi, (q0, qw) in enumerate(SCHUNKS):
                    ps = cur[qi]
                    mx = act2.tile([128, 1], F32, tag="smmax")
                    nc.vector.tensor_reduce(mx[:qw], ps[:qw, :S],
                                            axis=mybir.AxisListType.X,
                                            op=ALU.max)
                    nc.vector.tensor_scalar_mul(mx[:qw], mx[:qw], -scale)
                    ex = act2.tile([128, S], F32, tag="smex")
                    nc.scalar.activation(ex[:qw], ps[:qw, :S], AF.Exp,
                                         bias=mx[:qw], scale=scale)
                    sm = act2.tile([128, 1], F32, tag="smsum")
                    nc.vector.tensor_reduce(sm[:qw], ex[:qw],
                                            axis=mybir.AxisListType.X,
                                            op=ALU.add)
                    rc = act2.tile([128, 1], F32, tag="smrcp")
                    nc.vector.reciprocal(rc[:qw], sm[:qw])
                    at = act2.tile([128, S], BF16, tag="smat")
                    nc.scalar.activation(at[:qw], ex[:qw], AF.Identity,
                                         bias=zero_col[:qw], scale=rc[:qw])
                    for ki, (k0, kw) in enumerate(SCHUNKS):
                        pt = tp.tile([128, 128], BF16, tag="tp")
                        nc.tensor.transpose(pt[:kw, :qw], at[:qw, k0:k0 + kw],
                                            ident_bf[:qw, :qw])
                        nc.vector.tensor_copy(attnT[:kw, ki, q0:q0 + qw],
                                              pt[:kw, :qw])
                for ki, (k0, kw) in enumerate(SCHUNKS):
                    nc.tensor.matmul(pav[:64, :S],
                                     vT[:kw, ki, 64 * head:64 * head + 64],
                                     attnT[:kw, ki, :], start=(ki == 0),
                                     stop=(ki == 1))
                nc.scalar.activation(ob[hoff:hoff + 64, hc, :], pav[:64, :S],
                                     AF.Copy)

            for m in range(KC):
                ps = mm.tile([128, 512], F32, tag="mm")
                for kc in range(KC):
                    nc.tensor.matmul(ps[:, :S],
                                     wo[:, kc, 128 * m:128 * (m + 1)],
                                     ob[:, kc, :], start=(kc == 0),
                                     stop=(kc == KC - 1))
                tmp = act2.tile([128, S], F32, tag="otmp")
                nc.scalar.activation(tmp[:], ps[:, :S], AF.Identity,
                                     bias=lnp["bo"][:, l, m:m + 1], scale=1.0)
                nc.vector.tensor_tensor(h[:, m, :], h[:, m, :], tmp[:],
                                        ALU.add)

            # ---- MoE ----
            NQ = int(os.environ.get("VIT_NQ", "4"))
            HQ = HC // NQ  # 6 hidden chunks per quarter

            def gate_masks(lhs_src, soff, selrow):
                """Gate + top-1 masks for the image whose y32 columns sit at
                lhs_src[:, kc, soff:soff+S]; writes sel*gv rows (my experts
                only under EP, all under dense) into selrow[e]."""
                for si, (s0, sw) in enumerate(SCHUNKS):
                    pg = mm.tile([128, 512], F32, tag="mm")
                    for kc in range(KC):
                        nc.tensor.matmul(
                            pg[:sw, :E],
                            lhs_src[:, kc, soff + s0:soff + s0 + sw],
                            gw[:, kc, :], start=(kc == 0), stop=False)
                    nc.tensor.matmul(pg[:sw, :E], ones_row_f32[:, :sw],
                                     gb_row_sb[:, l, :], start=False,
                                     stop=True)
                    lg = act2.tile([128, E], F32, tag="glogit")
                    nc.vector.tensor_copy(lg[:sw], pg[:sw, :E])
                    mx = act2.tile([128, 1], F32, tag="gmax")
                    nc.vector.tensor_reduce(mx[:sw], lg[:sw],
                                            axis=mybir.AxisListType.X,
                                            op=ALU.max)
                    nmx = act2.tile([128, 1], F32, tag="gnmax")
                    nc.vector.tensor_scalar_mul(nmx[:sw], mx[:sw], -1.0)
                    ex = act2.tile([128, E], F32, tag="gex")
                    nc.scalar.activation(ex[:sw], lg[:sw], AF.Exp,
                                         bias=nmx[:sw], scale=1.0)
                    se = act2.tile([128, 1], F32, tag="gsum")
                    nc.vector.tensor_reduce(se[:sw], ex[:sw],
                                            axis=mybir.AxisListType.X,
                                            op=ALU.add)
                    gvc = act2.tile([128, 1], F32, tag="gvc")
                    nc.vector.reciprocal(gvc[:sw], se[:sw])
                    # selection masks with first-wins tie-break
                    asm = act2.tile([128, E + 1], F32, tag="gasm")
                    run = act2.tile([128, 1], F32, tag="grun")
                    nc.vector.memset(run[:sw], 0.0)
                    for e in range(E):
                        eq = act2.tile([128, 1], F32, tag="geq")
                        nc.vector.tensor_tensor(eq[:sw], lg[:sw, e:e + 1],
                                                mx[:sw], ALU.is_equal)
                        notrun = act2.tile([128, 1], F32, tag="gnr")
                        nc.vector.tensor_scalar(notrun[:sw], run[:sw], -1.0,
                                                1.0, ALU.mult, ALU.add)
                        nc.vector.tensor_tensor(asm[:sw, e:e + 1], eq[:sw],
                                                notrun[:sw], ALU.mult)
                        nc.vector.tensor_tensor(run[:sw], run[:sw],
                                                asm[:sw, e:e + 1], ALU.add)
                    for e in range(len(selrow)):
                        nc.vector.tensor_tensor(asm[:sw, e:e + 1],
                                                asm[:sw, e:e + 1], gvc[:sw],
                                                ALU.mult)
                    # transpose each needed column to a row at partition 0
                    for e in range(len(selrow)):
                        ptx = tp.tile([128, 128], F32, tag="tp", name="ptx")
                        nc.tensor.transpose(ptx[:1, :sw], asm[:sw, e:e + 1],
                                            ident_f32[:sw, :sw])
                        nc.vector.tensor_copy(selrow[e][:, s0:s0 + sw],
                                              ptx[:1, :sw])

            def expert_ffn(e, yb, ws, pselb, moeacc, first):
                """One expert's FFN over yb [128, KC, ws] bf16, masked by
                pselb [128, ws], accumulated into moeacc [128, KC, ws]."""
                for q in range(NQ):
                    w1q = wmoe.tile([128, KC, HID // NQ], BF16, tag="w1q")
                    nc.sync.dma_start(
                        w1q[:],
                        (g["w1"][l][e][:, :, q * (HID // NQ):
                                       (q + 1) * (HID // NQ)] if EP else
                         g["w1"][l][:, e, :, q * (HID // NQ):
                                    (q + 1) * (HID // NQ)]))
                    w2q = wmoe.tile([128, HQ, D], BF16, tag="w2q")
                    nc.sync.dma_start(
                        w2q[:],
                        (g["w2"][l][e][:, q * HQ:(q + 1) * HQ, :] if EP else
                         g["w2"][l][:, e, q * HQ:(q + 1) * HQ, :]))
                    hid = act2.tile([128, HQ, S2 if EP else S], BF16,
                                    tag="hid")
                    for m in range(HQ):
                        ps = mm.tile([128, 512], F32, tag="mm")
                        for kc in range(KC):
                            nc.tensor.matmul(
                                ps[:, :ws], w1q[:, kc, 128 * m:128 * (m + 1)],
                                yb[:, kc, :], start=(kc == 0),
                                stop=(kc == KC - 1))
                        nc.scalar.activation(
                            hid[:, m, :ws], ps[:, :ws], AF.Gelu_apprx_tanh,
                            bias=lnp["b1b"][:, l, e,
                                            q * HQ + m:q * HQ + m + 1],
                            scale=1.0)
                    for m in range(KC):
                        ps = mm.tile([128, 512], F32, tag="mm")
                        for kc in range(HQ):
                            last = (q < NQ - 1 and kc == HQ - 1)
                            nc.tensor.matmul(
                                ps[:, :ws], w2q[:, kc, 128 * m:128 * (m + 1)],
                                hid[:, kc, :ws], start=(kc == 0), stop=last)
                        if q == NQ - 1:
                            # add b2 for every token; drain scale by selgv
                            # zeroes it for unselected tokens
                            nc.tensor.matmul(
                                ps[:, :ws],
                                b2row_sb[:, e, 128 * m:128 * (m + 1)],
                                ones_row_S[:, :ws], start=False, stop=True)
                        tmp = act2.tile([128, S2 if EP else S], F32,
                                        tag="moedr")
                        nc.vector.tensor_tensor(tmp[:, :ws], ps[:, :ws],
                                                pselb, ALU.mult)
                        if first and q == 0:
                            nc.vector.tensor_copy(moeacc[:, m, :],
                                                  tmp[:, :ws])
                        else:
                            nc.vector.tensor_tensor(moeacc[:, m, :],
                                                    moeacc[:, m, :],
                                                    tmp[:, :ws], ALU.add)

            if EP:
                y32 = act.tile([128, KC, S], F32, tag="y32")
                layernorm(l, "ln2g", "ln2b", None, y32)
                ybounce = dram2.tile([128, KC, S], F32, tag="yb",
                                     name="ybounce")
                nc.sync.dma_start(ybounce[:], y32[:])
                ypair = dram2.tile([2, 128, KC, S], F32, tag="yp",
                                   name="ypair")
                nc.gpsimd.collective_compute(
                    "AllGather", ALU.bypass, replica_groups=GPAIR,
                    ins=[ybounce[:].opt()], outs=[ypair[:].opt()])
                postA(l + 2)
                y32p = act.tile([128, KC, S2], F32, tag="y32p")
                for i in range(2):
                    nc.sync.dma_start(y32p[:, :, i * S:(i + 1) * S],
                                      ypair[i])
                y2 = act.tile([128, KC, S2], BF16, tag="y2")
                for kc in range(KC):
                    nc.vector.tensor_copy(y2[:, kc, :], y32p[:, kc, :])
                ones_row_S = act2.tile([1, S2], BF16, tag="onesS")
                nc.vector.memset(ones_row_S[:], 1.0)
                selgv = [[act2.tile([1, S], F32, tag=f"sel{i}{e}",
                                    name=f"sel{i}{e}") for e in range(EH)]
                         for i in range(2)]
                for i in range(2):
                    gate_masks(y32p, i * S, selgv[i])
                moe2 = act.tile([128, KC, S2], F32, tag="moe")
                for e in range(EH):
                    pselbt = mm.tile([128, 512], F32, tag="mm", name="pselb")
                    for i in range(2):
                        nc.tensor.matmul(pselbt[:, i * S:(i + 1) * S],
                                         ones_row_f32[:], selgv[i][e][:],
                                         start=True, stop=True)
                    pselb_sb = act2.tile([128, S2], F32, tag="pselb_sb")
                    nc.vector.tensor_copy(pselb_sb[:], pselbt[:, :S2])
                    expert_ffn(e, y2, S2, pselb_sb[:], moe2, e == 0)
                moebf = act2.tile([128, KC, S2], BF16, tag="hid", name="moebf")
                for kc in range(KC):
                    nc.vector.tensor_copy(moebf[:, kc, :], moe2[:, kc, :])
                moebounce = dram2.tile([2, 128, KC, S], BF16, tag="mb",
                                       name="moebounce")
                for i in range(2):
                    nc.sync.dma_start(moebounce[i],
                                      moebf[:, :, i * S:(i + 1) * S])
                moeout = dram2.tile([128, KC, S], BF16, tag="mo",
                                    name="moeout")
                nc.gpsimd.collective_compute(
                    "ReduceScatter", ALU.add, replica_groups=GPAIR,
                    ins=[moebounce[:].opt()], outs=[moeout[:].opt()])
                postB(l + 2)
                moesb = act2.tile([128, KC, S], BF16, tag="moesb")
                nc.sync.dma_start(moesb[:], moeout[:])
                moes32 = act2.tile([128, KC, S], F32, tag="moes32")
                for kc in range(KC):
                    nc.vector.tensor_copy(moes32[:, kc, :], moesb[:, kc, :])
                for m in range(KC):
                    nc.vector.tensor_tensor(h[:, m, :], h[:, m, :],
                                            moes32[:, m, :], ALU.add)
            else:
                y = act.tile([128, KC, S], BF16, tag="y")
                y32 = act.tile([128, KC, S], F32, tag="y32")
                layernorm(l, "ln2g", "ln2b", y, y32)
                ones_row_S = act2.tile([1, S], BF16, tag="onesS")
                nc.vector.memset(ones_row_S[:], 1.0)
                selgv = [act2.tile([1, S], F32, tag=f"sel{e}",
                                   name=f"sel{e}") for e in range(E)]
                gate_masks(y32, 0, selgv)
                moe = act.tile([128, KC, S], F32, tag="moe")
                for e in range(E):
                    pselbt = mm.tile([128, 512], F32, tag="mm", name="pselb")
                    nc.tensor.matmul(pselbt[:, :S], ones_row_f32[:],
                                     selgv[e][:], start=True, stop=True)
                    pselb_sb = act2.tile([128, S], F32, tag="pselb_sb")
                    nc.vector.tensor_copy(pselb_sb[:], pselbt[:, :S])
                    expert_ffn(e, y, S, pselb_sb[:], moe, e == 0)
                for m in range(KC):
                    nc.vector.tensor_tensor(h[:, m, :], h[:, m, :],
                                            moe[:, m, :], ALU.add)

        # ---- head (cls token only) ----
        head_b_sb = act.tile([1, NC], F32, tag="headb")
        nc.sync.dma_start(head_b_sb[:], t["head_b"][:])
        cls_bf = act.tile([128, KC], BF16, tag="clsbf")
        nc.vector.tensor_copy(cls_bf[:], h[:, :, 0])
        logit = act.tile([1, NC], F32, tag="headout")
        for i in range(4):
            n0, nw = 250 * i, 250
            hw_sb = wmoe.tile([128, KC, nw], BF16, tag="w2q", name="hw_sb")
            nc.sync.dma_start(hw_sb[:], g["hw"][:, :, n0:n0 + nw])
            ps = mm.tile([128, 512], F32, tag="mm")
            for kc in range(KC):
                nc.tensor.matmul(ps[:1, :nw], cls_bf[:, kc:kc + 1],
                                 hw_sb[:, kc, :], start=(kc == 0),
                                 stop=(kc == KC - 1))
            nc.vector.tensor_tensor(logit[:, n0:n0 + nw], ps[:1, :nw],
                                    head_b_sb[:, n0:n0 + nw], ALU.add)
        nc.sync.dma_start(t["out"][:], logit[:])
        if t["hdbg"] is not None:
            nc.sync.dma_start(t["hdbg"][:], h[:])


def _prep(inputs):
    bf = ml_dtypes.bfloat16
    f32 = np.float32
    shared = {}
    big = {}
    # patch embedding weight: [D, CIN, P, P] -> [CIN*P*P, D] -> [128, KC, D]
    pw = inputs["patch_w"].reshape(D, CIN * PATCH * PATCH).T
    big["pwT"] = np.ascontiguousarray(
        pw.reshape(KC, 128, D).transpose(1, 0, 2)).astype(bf)
    shared["patch_b"] = np.ascontiguousarray(
        inputs["patch_b"].reshape(KC, 128).T).astype(f32)
    pos = inputs["pos_embed"][0] + np.concatenate(
        [inputs["cls_token"][0], np.zeros((NPAT, D), f32)], 0)  # [S, D]
    big["posT"] = np.ascontiguousarray(
        pos.T.reshape(KC, 128, S).transpose(1, 0, 2)).astype(f32)

    w = inputs["attn_in_w"]  # [12, 3D, D]
    big["wqkvT"] = np.ascontiguousarray(
        w.transpose(0, 2, 1).reshape(NDEPTH, KC, 128, 3 * D)
        .transpose(0, 2, 1, 3)).astype(bf)  # [12, 128, KC, 3D]
    shared["bqkv"] = np.ascontiguousarray(
        inputs["attn_in_b"].reshape(NDEPTH, 18, 128).transpose(2, 0, 1)
    ).astype(f32)
    shared["bqkv_row"] = inputs["attn_in_b"].reshape(
        1, NDEPTH, 3 * D).astype(bf)
    big["woT"] = np.ascontiguousarray(
        inputs["attn_out_w"].transpose(0, 2, 1).reshape(NDEPTH, KC, 128, D)
        .transpose(0, 2, 1, 3)).astype(bf)  # [12, 128, KC, D]
    shared["bo"] = np.ascontiguousarray(
        inputs["attn_out_b"].reshape(NDEPTH, KC, 128).transpose(2, 0, 1)
    ).astype(f32)
    for src, dst in (("ln1_g", "ln1g"), ("ln1_b", "ln1b"),
                     ("ln2_g", "ln2g"), ("ln2_b", "ln2b")):
        shared[dst] = np.ascontiguousarray(
            inputs[src].reshape(NDEPTH, KC, 128).transpose(2, 0, 1)
        ).astype(f32)
    shared["gwT32"] = np.ascontiguousarray(
        inputs["gate_w"].transpose(0, 2, 1).reshape(NDEPTH, KC, 128, E)
        .transpose(2, 0, 1, 3)).astype(f32)
    shared["gb_row"] = inputs["gate_b"].reshape(1, NDEPTH, E).astype(f32)
    if EP:
        # expert-major per-layer layout so a contiguous half of the flat
        # buffer is exactly two experts (gathered by the 4-core groups)
        big["w1T"] = np.ascontiguousarray(
            inputs["w1"].reshape(NDEPTH, E, KC, 128, HID)
            .transpose(0, 1, 3, 2, 4)).astype(bf)  # [12, E, 128, KC, HID]
        big["w2T"] = np.ascontiguousarray(
            inputs["w2"].reshape(NDEPTH, E, HC, 128, D)
            .transpose(0, 1, 3, 2, 4)).astype(bf)  # [12, E, 128, HC, D]
    else:
        big["w1T"] = np.ascontiguousarray(
            inputs["w1"].reshape(NDEPTH, E, KC, 128, HID)
            .transpose(0, 3, 1, 2, 4)).astype(bf)  # [12, 128, E, KC, HID]
        big["w2T"] = np.ascontiguousarray(
            inputs["w2"].reshape(NDEPTH, E, HC, 128, D)
            .transpose(0, 3, 1, 2, 4)).astype(bf)  # [12, 128, E, HC, D]
    shared["b1b"] = np.ascontiguousarray(
        inputs["b1"].reshape(NDEPTH, E, HC, 128).transpose(3, 0, 1, 2)
    ).astype(f32)
    shared["b2row"] = inputs["b2"].reshape(1, NDEPTH, E, D).astype(bf)
    big["hwT"] = np.ascontiguousarray(
        inputs["head_w"].T.reshape(KC, 128, NC).transpose(1, 0, 2)).astype(bf)
    shared["head_b"] = inputs["head_b"].reshape(1, NC).astype(f32)

    # per-core image patches, feature-major [128, KC, NPAT]
    x = inputs["x"]  # [B, CIN, IMG, IMG]
    xp = x.reshape(B, CIN, 14, PATCH, 14, PATCH).transpose(0, 1, 3, 5, 2, 4)
    xp = xp.reshape(B, CIN * PATCH * PATCH, NPAT)  # [B, 768, 196]
    xps = [np.ascontiguousarray(
        xp[b].reshape(KC, 128, NPAT).transpose(1, 0, 2)).astype(bf)
        for b in range(B)]
    return shared, big, xps


def _shard_big(big, c):
    """Per-core 1/8 contiguous shard of each replicated weight tensor."""
    out = {}
    if GATHER:
        for name, key, per_layer in (("wqkvT", "wqkvT_s", True),
                                     ("woT", "woT_s", True),
                                     ("w1T", "w1T_s", True),
                                     ("w2T", "w2T_s", True),
                                     ("pwT", "pwT_s", False),
                                     ("posT", "posT_s", False),
                                     ("hwT", "hwT_s", False)):
            a = big[name]
            if per_layer:
                nrow = NDEPTH if EP else NDEPTH // 2
                flat = a.reshape(nrow, -1)
                n8 = flat.shape[1] // N_CORES
                out[key] = np.ascontiguousarray(
                    flat[:, c * n8:(c + 1) * n8])
            else:
                flat = a.reshape(-1)
                n8 = flat.shape[0] // N_CORES
                out[key] = np.ascontiguousarray(flat[c * n8:(c + 1) * n8])
    else:
        for name in ("wqkvT", "woT", "w1T", "w2T"):
            out[name + "_g"] = big[name]
        for name in ("pwT", "posT", "hwT"):
            out[name + "_g"] = big[name]
    return out


def kernel(**inputs):
    inputs = {k: np.asarray(v) for k, v in inputs.items()}
    shared, big, xps = _prep(inputs)
    nc = _build(DEPTH)
    in_maps = []
    for c in range(N_CORES):
        m = dict(shared, xp=xps[c], **_shard_big(big, c))
        if EP:
            # core-resident expert half: biases sliced, gate columns
            # permuted so this core's experts are always columns 0..EH-1
            half = int(c >= 4)
            perm = [0, 1, 2, 3] if half == 0 else [2, 3, 0, 1]
            m["b1b"] = np.ascontiguousarray(
                shared["b1b"][:, :, 2 * half:2 * half + EH])
            m["b2row"] = np.ascontiguousarray(
                shared["b2row"][:, :, 2 * half:2 * half + EH])
            m["gwT32"] = np.ascontiguousarray(shared["gwT32"][..., perm])
            m["gb_row"] = np.ascontiguousarray(shared["gb_row"][..., perm])
        in_maps.append(m)
    trace = bool(int(os.environ.get("VIT_TRACE", "0")))
    res = run_bass_kernel_spmd(nc, in_maps, list(range(N_CORES)),
                               trace=trace)
    kernel._nc = nc
    kernel._in_maps = in_maps
    kernel._decls = {k: v for k, v in _LAST_DECLS.items() if k != "out"}
    out = np.stack([res.results[c]["out"][0] for c in range(N_CORES)])
    if DEBUG_H:
        kernel._hdbg = [res.results[c]["hdbg"] for c in range(N_CORES)]
    kernel._res = res
    return out.astype(np.float32)


# revision 22
# speedup vs baseline: 1.0302x; 1.0302x over previous
# MoE-ViT forward pass on 8 trn2 NeuronCores.
# Attention is data-parallel (1 image/core); the MoE MLP is pair
# expert-parallel: cores 0-3 hold experts {0,1}, cores 4-7 hold {2,3}, and
# pair (c, c+4) swaps LN2 activations (pair AllGather), each computing its
# two experts dense-masked over both images, then pair-ReduceScatter(add)
# returns each image's combined MoE output to its home core.  Gate columns
# are host-permuted per core so "my experts" are always columns 0..1;
# routing math stays fp32 and bit-identical to the data-parallel version.
# Layout: activations feature-major [D on partitions (6x128), tokens on free].
# Matmuls in bf16 (fp32 PSUM accumulation), LN/softmax/routing math in fp32.
# Weights are identical on every core, so each core receives only a 1/8
# shard over the (slow) host link; full tensors are reassembled on device
# with AllGather collectives (4-core groups for the expert halves) that are
# posted two layers ahead of use, interleaved with the pair exchanges since
# collectives execute strictly in posted order.
import os
import numpy as np
import ml_dtypes

import concourse.bass as bass
import concourse.bacc as bacc
import concourse.mybir as mybir
import concourse.tile as tile
from concourse.tile import ScopedClock
from concourse.bass_utils import run_bass_kernel_spmd
from concourse.masks import make_identity
import bass_rust

F32 = mybir.dt.float32
BF16 = mybir.dt.bfloat16
AF = mybir.ActivationFunctionType
ALU = mybir.AluOpType

B, IMG, PATCH, CIN = 8, 224, 16, 3
D, NH, E, HID, NDEPTH, NC = 768, 12, 4, 3072, 12, 1000
NPAT = (IMG // PATCH) ** 2  # 196
S = NPAT + 1  # 197
HD = D // NH  # 64
EPS = 1e-5
KC = D // 128  # 6 feature chunks
HC = HID // 128  # 24 hidden chunks
HCH = HC // 2  # 12 hidden chunks per half
N_CORES = 8
DEPTH = int(os.environ.get("VIT_DEPTH", NDEPTH))
DEBUG_H = bool(int(os.environ.get("VIT_DEBUG_H", "0")))
GATHER = bool(int(os.environ.get("VIT_GATHER", "1")))
COMPUTE = bool(int(os.environ.get("VIT_COMPUTE", "1")))
# pair expert-parallel MoE: cores 0-3 own experts {0,1}, cores 4-7 own
# {2,3}; pair (c, c+4) exchanges activations and splits the expert work
# for its two images.  Requires GATHER.
EP = bool(int(os.environ.get("VIT_EP", "0"))) and GATHER
EH = E // 2  # experts resident per core under EP
S2 = 2 * S  # tokens of the two pair images, concatenated
GFULL = [list(range(N_CORES))]
G4 = [[0, 1, 2, 3], [4, 5, 6, 7]]  # expert-half gather groups
GPAIR = [[0, 4], [1, 5], [2, 6], [3, 7]]  # image-pair exchange groups

_LAST_DECLS = {}
SCHUNKS = [(0, 128), (128, S - 128)]  # token chunks (start, width)
NCHUNKS = [(0, 512), (512, 256)]  # dout free-dim chunks for token-major mm

# elements per core-shard of each gathered tensor ( / layer for per-layer)
SH_QKV = 128 * KC * 3 * D // N_CORES  # 221184
SH_WO = 128 * KC * D // N_CORES  # 73728
SH_W1 = 128 * E * KC * HID // N_CORES  # 1179648
SH_W2 = 128 * E * HC * D // N_CORES  # 1179648
SH_PW = 128 * KC * D // N_CORES
SH_POS = 128 * KC * S // N_CORES
SH_HW = 128 * KC * NC // N_CORES


def _patched_drain(self, tick_clock, wait_clock):
    # Upstream _drain_and_barrier puts every outstanding proc's sem wait on
    # the single Drain instruction; walrus CoreV3 codegen caps inline sync
    # waits below that. Split: one carrier nop per proc, each with one wait.
    gc_vals = eval(repr(tick_clock.global_clock)[len("VectorClock("):-1])
    for proc, _handle in self.sems.allocated().items():
        single = bass_rust.VectorClock(
            [v if i == proc else 0 for i, v in enumerate(gc_vals)])
        nop = self.nc.sync.nop(nofuse=True)
        wait_clock.add_sem_waits(nop.ins, ScopedClock({None: single}))
    self.nc.sync.drain()
    self.nc.all_engine_barrier()
    popped = self.nc._tile_sem_poison_stack.pop()
    assert popped is self._sem_poison
    self.nc.clear_and_free_semaphores(list(self.sems.allocated().values()))
    self.nc.all_engine_barrier()


tile.TileContext._drain_and_barrier = _patched_drain


def _build(depth):
    nc = bacc.Bacc("TRN2", target_bir_lowering=False, debug=False,
                   num_devices=N_CORES)

    def din(name, shape, dt=F32):
        return nc.dram_tensor(name, shape, dt, kind="ExternalInput").ap()

    t = {}
    t["xp"] = din("xp", [128, KC, NPAT], BF16)
    t["patch_b"] = din("patch_b", [128, KC])
    t["bqkv"] = din("bqkv", [128, NDEPTH, 18])
    t["bqkv_row"] = din("bqkv_row", [1, NDEPTH, 3 * D], BF16)
    t["bo"] = din("bo", [128, NDEPTH, KC])
    t["ln1g"] = din("ln1g", [128, NDEPTH, KC])
    t["ln1b"] = din("ln1b", [128, NDEPTH, KC])
    t["ln2g"] = din("ln2g", [128, NDEPTH, KC])
    t["ln2b"] = din("ln2b", [128, NDEPTH, KC])
    t["gwT32"] = din("gwT32", [128, NDEPTH, KC, E])
    t["gb_row"] = din("gb_row", [1, NDEPTH, E])
    EB = EH if EP else E
    t["b1b"] = din("b1b", [128, NDEPTH, EB, HC])
    t["b2row"] = din("b2row", [1, NDEPTH, EB, D], BF16)
    t["head_b"] = din("head_b", [1, NC])
    if GATHER:
        # per-core 1/8 shards of the replicated weights (layer-major)
        t["wqkvT_s"] = din("wqkvT_s", [NDEPTH, SH_QKV], BF16)
        t["woT_s"] = din("woT_s", [NDEPTH, SH_WO], BF16)
        t["w1T_s"] = din("w1T_s", [NDEPTH, SH_W1], BF16)
        t["w2T_s"] = din("w2T_s", [NDEPTH, SH_W2], BF16)
        t["pwT_s"] = din("pwT_s", [SH_PW], BF16)
        t["posT_s"] = din("posT_s", [SH_POS])
        t["hwT_s"] = din("hwT_s", [SH_HW], BF16)
    else:
        t["wqkvT_g"] = din("wqkvT_g", [NDEPTH, 128, KC, 3 * D], BF16)
        t["woT_g"] = din("woT_g", [NDEPTH, 128, KC, D], BF16)
        t["w1T_g"] = din("w1T_g", [NDEPTH, 128, E, KC, HID], BF16)
        t["w2T_g"] = din("w2T_g", [NDEPTH, 128, E, HC, D], BF16)
        t["pwT_g"] = din("pwT_g", [128, KC, D], BF16)
        t["posT_g"] = din("posT_g", [128, KC, S])
        t["hwT_g"] = din("hwT_g", [128, KC, NC], BF16)

    t["out"] = nc.dram_tensor("out", [1, NC], F32, kind="ExternalOutput").ap()
    _LAST_DECLS.clear()
    _LAST_DECLS.update({k: v for k, v in t.items() if v is not None and k != "out"})
    t["hdbg"] = None
    if DEBUG_H:
        t["hdbg"] = nc.dram_tensor("hdbg", [128, KC, S], F32,
                                   kind="ExternalOutput").ap()

    with tile.TileContext(nc) as tc:
        _emit(nc, tc, depth, t)
    nc.finalize()
    return nc


def _emit_gathers(nc, tc, ctx, depth, t):
    """Weight distribution via AllGather from per-core shards.

    Collectives execute strictly in posted order, so the big weight
    gathers are NOT all posted up front: layers 0/1 (plus the small
    shared tensors) are posted at startup and the layer loop posts layer
    l+2's gathers mid-layer, keeping the CC queue drained ahead of the
    latency-critical pair exchanges (EP mode).  Returns (g, postA, postB)
    where postA(l)/postB(l) post the two halves of layer l's gathers."""
    if not GATHER:
        return ({"qkv": [t["wqkvT_g"][l] for l in range(NDEPTH)],
                 "wo": [t["woT_g"][l] for l in range(NDEPTH)],
                 "w1": [t["w1T_g"][l] for l in range(NDEPTH)],
                 "w2": [t["w2T_g"][l] for l in range(NDEPTH)],
                 "pw": t["pwT_g"], "pos": t["posT_g"], "hw": t["hwT_g"]},
                lambda l: None, lambda l: None)
    dram = ctx.enter_context(tc.tile_pool(name="gdram", bufs=1, space="DRAM"))

    def gather(name, src_ap, shard_el, full_shape, dt, groups):
        bounce = dram.tile([shard_el], dt, tag=f"b_{name}", name=f"b_{name}")
        nc.sync.dma_start(bounce[:], src_ap)
        shared = "Shared" if len(groups[0]) > 4 else "Local"
        out = dram.tile(list(full_shape), dt, addr_space=shared,
                        tag=f"g_{name}", name=f"g_{name}")
        nc.gpsimd.collective_compute(
            "AllGather", ALU.bypass, replica_groups=groups,
            ins=[bounce[:].opt()], outs=[out[:].opt()])
        return out

    g = {"qkv": [None] * NDEPTH, "wo": [None] * NDEPTH,
         "w1": [None] * NDEPTH, "w2": [None] * NDEPTH}

    def postA(l):
        if l >= depth or g["qkv"][l] is not None:
            return
        g["qkv"][l] = gather(f"qkv{l}", t["wqkvT_s"][l], SH_QKV,
                             [128, KC, 3 * D], BF16, GFULL)
        if EP:
            g["w1"][l] = gather(f"w1{l}", t["w1T_s"][l], SH_W1,
                                [EH, 128, KC, HID], BF16, G4)
        else:
            g["w1"][l] = gather(f"w1{l}", t["w1T_s"][l], SH_W1,
                                [128, E, KC, HID], BF16, GFULL)

    def postB(l):
        if l >= depth or g["wo"][l] is not None:
            return
        g["wo"][l] = gather(f"wo{l}", t["woT_s"][l], SH_WO,
                            [128, KC, D], BF16, GFULL)
        if EP:
            g["w2"][l] = gather(f"w2{l}", t["w2T_s"][l], SH_W2,
                                [EH, 128, HC, D], BF16, G4)
        else:
            g["w2"][l] = gather(f"w2{l}", t["w2T_s"][l], SH_W2,
                                [128, E, HC, D], BF16, GFULL)

    g["pw"] = gather("pw", t["pwT_s"][:], SH_PW, [128, KC, D], BF16, GFULL)
    g["pos"] = gather("pos", t["posT_s"][:], SH_POS, [128, KC, S], F32,
                      GFULL)
    for l in range(min(2, depth)):
        postA(l)
        postB(l)
    g["hw"] = gather("hw", t["hwT_s"][:], SH_HW, [128, KC, NC], BF16, GFULL)
    return g, postA, postB


def _emit(nc, tc, depth, t):
    import contextlib
    ctx = contextlib.ExitStack()
    with ctx:
        def _env(name, dflt):
            return int(os.environ.get(name, dflt))
        g, postA, postB = _emit_gathers(nc, tc, ctx, depth, t)
        if GATHER and COMPUTE:
            for l in range(2, depth):
                if not EP:
                    postA(l)
                    postB(l)
        if EP:
            dram2 = ctx.enter_context(
                tc.tile_pool(name="xdram", bufs=2, space="DRAM"))
        if not COMPUTE:
            # gathers-only benchmark mode: touch one gathered byte per tensor
            # so nothing is dead, write a dummy output
            for l in range(2, depth):
                postA(l)
                postB(l)
            sbp = ctx.enter_context(tc.tile_pool(name="sbp", bufs=1))
            acc = sbp.tile([1, NC], F32)
            nc.vector.memset(acc[:], 0.0)
            probes = [g["pw"], g["pos"], g["hw"]] + \
                [g[k][l] for k in ("qkv", "wo", "w1", "w2")
                 for l in range(depth)]
            for i, ap in enumerate(probes):
                pr = sbp.tile([1, 4], F32 if ap is g["pos"] else BF16,
                              tag="probe", bufs=2, name=f"probe{i}")
                if len(ap.shape) == 3:
                    src = ap[:1, 0, 0:4]
                elif EP:
                    src = ap[0, :1, 0, 0:4]
                else:
                    src = ap[:1, 0, 0, 0:4]
                nc.sync.dma_start(pr[:], src)
                pf = sbp.tile([1, 4], F32, tag="probef", bufs=2,
                              name=f"probef{i}")
                nc.vector.tensor_copy(pf[:], pr[:])
                nc.vector.tensor_tensor(acc[:, 0:4], acc[:, 0:4], pf[:],
                                        ALU.add)
            nc.sync.dma_start(t["out"][:], acc[:])
            return
        state = ctx.enter_context(tc.tile_pool(name="state", bufs=1))
        wpool = ctx.enter_context(
            tc.tile_pool(name="wpool", bufs=_env("VIT_WPOOL", 1)))
        wmoe = ctx.enter_context(
            tc.tile_pool(name="wmoe", bufs=_env("VIT_WMOE", 2)))
        act = ctx.enter_context(tc.tile_pool(name="act", bufs=1))
        act2 = ctx.enter_context(
            tc.tile_pool(name="act2", bufs=_env("VIT_ACT2", 2)))
        mm = ctx.enter_context(
            tc.tile_pool(name="mm", bufs=_env("VIT_MM", 6), space="PSUM"))
        stat = ctx.enter_context(tc.tile_pool(name="stat", bufs=1,
                                              space="PSUM"))
        tp = ctx.enter_context(
            tc.tile_pool(name="tp", bufs=_env("VIT_TP", 2), space="PSUM"))

        # ---- persistent small tiles ----
        ones_col_f32 = state.tile([128, 1], F32)  # lhsT for partition sums
        nc.vector.memset(ones_col_f32[:], 1.0)
        ones_row_f32 = state.tile([1, 128], F32)  # lhsT for bcast (K=1)
        nc.vector.memset(ones_row_f32[:], 1.0)
        ones_row_bf = state.tile([1, 128], BF16)
        nc.vector.memset(ones_row_bf[:], 1.0)
        ident_bf = state.tile([128, 128], BF16)
        make_identity(nc, ident_bf[:])
        ident_f32 = state.tile([128, 128], F32)
        make_identity(nc, ident_f32[:])
        eps_t = state.tile([1, 1], F32)
        nc.vector.memset(eps_t[:], EPS)
        zero_col = state.tile([128, 1], F32)
        nc.vector.memset(zero_col[:], 0.0)

        h = state.tile([128, KC, S], F32)
        nc.sync.dma_start(h[:], g["pos"][:])

        lnp = {}
        for name in ("ln1g", "ln1b", "ln2g", "ln2b", "bqkv", "bo", "patch_b",
                     "b1b"):
            lnp[name] = state.tile(list(t[name].shape), F32, name=name)
            nc.sync.dma_start(lnp[name][:], t[name][:])
        gb_row_sb = state.tile([1, NDEPTH, E], F32)
        nc.sync.dma_start(gb_row_sb[:], t["gb_row"][:])

        # ---- patch embedding: h[:, m, 1:] += pwT.T @ xp + patch_b ----
        xp_sb = wmoe.tile([128, KC, NPAT], BF16, tag="w1q", name="xp_sb")
        nc.sync.dma_start(xp_sb[:], t["xp"][:])
        pw_sb = wmoe.tile([128, KC, D], BF16, tag="w2q", name="pw_sb")
        nc.sync.dma_start(pw_sb[:], g["pw"][:])
        for m in range(KC):
            ps = mm.tile([128, 512], F32, tag="mm")
            for kc in range(KC):
                nc.tensor.matmul(ps[:, :NPAT],
                                 pw_sb[:, kc, 128 * m:128 * (m + 1)],
                                 xp_sb[:, kc, :], start=(kc == 0),
                                 stop=(kc == KC - 1))
            tmp = act2.tile([128, NPAT], F32, tag="embtmp")
            nc.scalar.activation(tmp[:], ps[:, :NPAT], AF.Identity,
                                 bias=lnp["patch_b"][:, m:m + 1], scale=1.0)
            nc.vector.tensor_tensor(h[:, m, 1:S], h[:, m, 1:S], tmp[:],
                                    ALU.add)

        def layernorm(l, gk, bk, out_bf16, out_f32=None):
            # h [128, KC, S] fp32 -> out_bf16 [128, KC, S] = norm(h)*g + b
            hsq = act.tile([128, KC, S], F32, tag="hsq")
            for kc in range(KC):
                nc.vector.tensor_tensor(hsq[:, kc, :], h[:, kc, :],
                                        h[:, kc, :], ALU.mult)
            mean = act.tile([1, S], F32, tag="lnmean")
            em2 = act.tile([1, S], F32, tag="lnem2")
            for dst, src in ((mean, h), (em2, hsq)):
                pstat = mm.tile([128, 512], F32, tag="mm", name="pstat")
                for kc in range(KC):
                    nc.tensor.matmul(pstat[:1, :S], ones_col_f32[:],
                                     src[:, kc, :], start=(kc == 0),
                                     stop=(kc == KC - 1))
                nc.vector.tensor_scalar_mul(dst[:], pstat[:1, :S], 1.0 / D)
            var = act.tile([1, S], F32, tag="lnvar")
            nc.vector.tensor_tensor(var[:], mean[:], mean[:], ALU.mult)
            nc.vector.tensor_tensor(var[:], em2[:], var[:], ALU.subtract)
            sd = act.tile([1, S], F32, tag="lnsd")
            nc.scalar.activation(sd[:], var[:], AF.Sqrt, bias=eps_t[:], scale=1.0)
            rstd = act.tile([1, S], F32, tag="lnrstd")
            nc.vector.reciprocal(rstd[:], sd[:])
            nmr = act.tile([1, S], F32, tag="lnnmr")
            nc.vector.tensor_tensor(nmr[:], mean[:], rstd[:], ALU.mult)
            nc.vector.tensor_scalar_mul(nmr[:], nmr[:], -1.0)
            pb1t = mm.tile([128, 512], F32, tag="mm", name="pb1")
            pb1 = pb1t[:, :S]
            nc.tensor.matmul(pb1, ones_row_f32[:], rstd[:], start=True,
                             stop=True)
            pb2t = mm.tile([128, 512], F32, tag="mm", name="pb2")
            pb2 = pb2t[:, :S]
            nc.tensor.matmul(pb2, ones_row_f32[:], nmr[:], start=True,
                             stop=True)
            for kc in range(KC):
                tmp = act2.tile([128, S], F32, tag="lntmp")
                nc.vector.tensor_tensor(tmp[:], h[:, kc, :], pb1, ALU.mult)
                nc.vector.tensor_tensor(tmp[:], tmp[:], pb2, ALU.add)
                if out_f32 is not None:
                    nc.scalar.activation(out_f32[:, kc, :], tmp[:],
                                         AF.Identity,
                                         bias=lnp[bk][:, l, kc:kc + 1],
                                         scale=lnp[gk][:, l, kc:kc + 1])
                    if out_bf16 is not None:
                        nc.vector.tensor_copy(out_bf16[:, kc, :],
                                              out_f32[:, kc, :])
                else:
                    nc.scalar.activation(out_bf16[:, kc, :], tmp[:],
                                         AF.Identity,
                                         bias=lnp[bk][:, l, kc:kc + 1],
                                         scale=lnp[gk][:, l, kc:kc + 1])

        for l in range(depth):
            wqkv = wpool.tile([128, KC, 3 * D], BF16, tag="wqkv")
            nc.sync.dma_start(wqkv[:], g["qkv"][l][:])
            wo = wpool.tile([128, KC, D], BF16, tag="wo")
            nc.sync.dma_start(wo[:], g["wo"][l][:])
            gw = wpool.tile([128, KC, E], F32, tag="gw")
            nc.sync.dma_start(gw[:], t["gwT32"][:, l])
            bqkv_row_sb = act2.tile([1, 3 * D], BF16, tag="bqkvr")
            nc.sync.dma_start(bqkv_row_sb[:], t["bqkv_row"][:, l])
            b2row_sb = act2.tile([1, EH if EP else E, D], BF16, tag="b2r")
            nc.sync.dma_start(b2row_sb[:], t["b2row"][:, l])

            # ---- attention ----
            qn = act.tile([128, KC, S], BF16, tag="qn")
            layernorm(l, "ln1g", "ln1b", qn)
            hbf = act.tile([128, KC, S], BF16, tag="hbf")
            for kc in range(KC):
                nc.vector.tensor_copy(hbf[:, kc, :], h[:, kc, :])

            qb = act.tile([128, KC, S], BF16, tag="qb")
            kb = act.tile([128, KC, S], BF16, tag="kb")
            for j, (src, dst) in enumerate(((qn, qb), (hbf, kb))):
                for m in range(KC):
                    ps = mm.tile([128, 512], F32, tag="mm")
                    for kc in range(KC):
                        nc.tensor.matmul(
                            ps[:, :S],
                            wqkv[:, kc, j * D + 128 * m:j * D + 128 * (m + 1)],
                            src[:, kc, :], start=(kc == 0),
                            stop=(kc == KC - 1))
                    nc.scalar.activation(
                        dst[:, m, :], ps[:, :S], AF.Identity,
                        bias=lnp["bqkv"][:, l, j * KC + m:j * KC + m + 1],
                        scale=1.0)
            # v token-major: vT [s-chunk partitions, 2, D]
            vT = act.tile([128, 2, D], BF16, tag="vT")
            for si, (s0, sw) in enumerate(SCHUNKS):
                for (n0, nw) in NCHUNKS:
                    ps = mm.tile([128, 512], F32, tag="mm")
                    for kc in range(KC):
                        nc.tensor.matmul(
                            ps[:sw, :nw], hbf[:, kc, s0:s0 + sw],
                            wqkv[:, kc, 2 * D + n0:2 * D + n0 + nw],
                            start=(kc == 0), stop=False)
                    nc.tensor.matmul(
                        ps[:sw, :nw], ones_row_bf[:, :sw],
                        bqkv_row_sb[:, 2 * D + n0:2 * D + n0 + nw],
                        start=False, stop=True)
                    nc.vector.tensor_copy(vT[:sw, si, n0:n0 + nw],
                                          ps[:sw, :nw])

            ob = act.tile([128, KC, S], BF16, tag="ob")
            scale = 1.0 / float(np.sqrt(HD))

            def _score(head):
                hc, hoff = head // 2, (head % 2) * 64
                q_h = qb[hoff:hoff + 64, hc, :]
                k_h = kb[hoff:hoff + 64, hc, :]
                tiles = []
                for qi, (q0, qw) in enumerate(SCHUNKS):
                    ps = mm.tile([128, 512], F32, tag="mm",
                                 name=f"ps_h{head}q{qi}")
                    nc.tensor.matmul(ps[:qw, :S], q_h[:, q0:q0 + qw], k_h,
                                     start=True, stop=True)
                    tiles.append(ps)
                return tiles

            pend = _score(0)
            for head in range(NH):
                hc, hoff = head // 2, (head % 2) * 64
                cur = pend
                if head + 1 < NH:
                    pend = _score(head + 1)
                pav = mm.tile([128, 512], F32, tag="mm")
                attnT = act2.tile([128, 2, S], BF16, tag="attnT")
                for qi, (q0, qw) in enumerate(SCHUNKS):
                    ps = cur[qi]
                    mx = act2.tile([128, 1], F32, tag="smmax")
                    nc.vector.tensor_reduce(mx[:qw], ps[:qw, :S],
                                            axis=mybir.AxisListType.X,
                                            op=ALU.max)
                    nc.vector.tensor_scalar_mul(mx[:qw], mx[:qw], -scale)
                    ex = act2.tile([128, S], F32, tag="smex")
                    nc.scalar.activation(ex[:qw], ps[:qw, :S], AF.Exp,
                                         bias=mx[:qw], scale=scale)
                    sm = act2.tile([128, 1], F32, tag="smsum")
                    nc.vector.tensor_reduce(sm[:qw], ex[:qw],
                                            axis=mybir.AxisListType.X,
                                            op=ALU.add)
                    rc = act2.tile([128, 1], F32, tag="smrcp")
                    nc.vector.reciprocal(rc[:qw], sm[:qw])
                    at = act2.tile([128, S], BF16, tag="smat")
                    nc.scalar.activation(at[:qw], ex[:qw], AF.Identity,
                                         bias=zero_col[:qw], scale=rc[:qw])
                    for ki, (k0, kw) in enumerate(SCHUNKS):
                        pt = tp.tile([128, 128], BF16, tag="tp")
                        nc.tensor.transpose(pt[:kw, :qw], at[:qw, k0:k0 + kw],
                                            ident_bf[:qw, :qw])
                        nc.vector.tensor_copy(attnT[:kw, ki, q0:q0 + qw],
                                              pt[:kw, :qw])
                for ki, (k0, kw) in enumerate(SCHUNKS):
                    nc.tensor.matmul(pav[:64, :S],
                                     vT[:kw, ki, 64 * head:64 * head + 64],
                                     attnT[:kw, ki, :], start=(ki == 0),
                                     stop=(ki == 1))
                nc.scalar.activation(ob[hoff:hoff + 64, hc, :], pav[:64, :S],
                                     AF.Copy)

            for m in range(KC):
                ps = mm.tile([128, 512], F32, tag="mm")
                for kc in range(KC):
                    nc.tensor.matmul(ps[:, :S],
                                     wo[:, kc, 128 * m:128 * (m + 1)],
                                     ob[:, kc, :], start=(kc == 0),
                                     stop=(kc == KC - 1))
                tmp = act2.tile([128, S], F32, tag="otmp")
                nc.scalar.activation(tmp[:], ps[:, :S], AF.Identity,
                                     bias=lnp["bo"][:, l, m:m + 1], scale=1.0)
                nc.vector.tensor_tensor(h[:, m, :], h[:, m, :], tmp[:],
                                        ALU.add)

            # ---- MoE ----
            NQ = int(os.environ.get("VIT_NQ", "4"))
            HQ = HC // NQ  # 6 hidden chunks per quarter

            def gate_masks(lhs_src, soff, selrow):
                """Gate + top-1 masks for the image whose y32 columns sit at
                lhs_src[:, kc, soff:soff+S]; writes sel*gv rows (my experts
                only under EP, all under dense) into selrow[e]."""
                for si, (s0, sw) in enumerate(SCHUNKS):
                    pg = mm.tile([128, 512], F32, tag="mm")
                    for kc in range(KC):
                        nc.tensor.matmul(
                            pg[:sw, :E],
                            lhs_src[:, kc, soff + s0:soff + s0 + sw],
                            gw[:, kc, :], start=(kc == 0), stop=False)
                    nc.tensor.matmul(pg[:sw, :E], ones_row_f32[:, :sw],
                                     gb_row_sb[:, l, :], start=False,
                                     stop=True)
                    lg = act2.tile([128, E], F32, tag="glogit")
                    nc.vector.tensor_copy(lg[:sw], pg[:sw, :E])
                    mx = act2.tile([128, 1], F32, tag="gmax")
                    nc.vector.tensor_reduce(mx[:sw], lg[:sw],
                                            axis=mybir.AxisListType.X,
                                            op=ALU.max)
                    nmx = act2.tile([128, 1], F32, tag="gnmax")
                    nc.vector.tensor_scalar_mul(nmx[:sw], mx[:sw], -1.0)
                    ex = act2.tile([128, E], F32, tag="gex")
                    nc.scalar.activation(ex[:sw], lg[:sw], AF.Exp,
                                         bias=nmx[:sw], scale=1.0)
                    se = act2.tile([128, 1], F32, tag="gsum")
                    nc.vector.tensor_reduce(se[:sw], ex[:sw],
                                            axis=mybir.AxisListType.X,
                                            op=ALU.add)
                    gvc = act2.tile([128, 1], F32, tag="gvc")
                    nc.vector.reciprocal(gvc[:sw], se[:sw])
                    # selection masks with first-wins tie-break
                    asm = act2.tile([128, E + 1], F32, tag="gasm")
                    run = act2.tile([128, 1], F32, tag="grun")
                    nc.vector.memset(run[:sw], 0.0)
                    for e in range(E):
                        eq = act2.tile([128, 1], F32, tag="geq")
                        nc.vector.tensor_tensor(eq[:sw], lg[:sw, e:e + 1],
                                                mx[:sw], ALU.is_equal)
                        notrun = act2.tile([128, 1], F32, tag="gnr")
                        nc.vector.tensor_scalar(notrun[:sw], run[:sw], -1.0,
                                                1.0, ALU.mult, ALU.add)
                        nc.vector.tensor_tensor(asm[:sw, e:e + 1], eq[:sw],
                                                notrun[:sw], ALU.mult)
                        nc.vector.tensor_tensor(run[:sw], run[:sw],
                                                asm[:sw, e:e + 1], ALU.add)
                    for e in range(len(selrow)):
                        nc.vector.tensor_tensor(asm[:sw, e:e + 1],
                                                asm[:sw, e:e + 1], gvc[:sw],
                                                ALU.mult)
                    # transpose each needed column to a row at partition 0
                    for e in range(len(selrow)):
                        ptx = tp.tile([128, 128], F32, tag="tp", name="ptx")
                        nc.tensor.transpose(ptx[:1, :sw], asm[:sw, e:e + 1],
                                            ident_f32[:sw, :sw])
                        nc.vector.tensor_copy(selrow[e][:, s0:s0 + sw],
                                              ptx[:1, :sw])

            def expert_ffn(e, yb, ws, pselb, moeacc, first):
                """One expert's FFN over yb [128, KC, ws] bf16, masked by
                pselb [128, ws], accumulated into moeacc [128, KC, ws]."""
                for q in range(NQ):
                    w1q = wmoe.tile([128, KC, HID // NQ], BF16, tag="w1q")
                    nc.sync.dma_start(
                        w1q[:],
                        (g["w1"][l][e][:, :, q * (HID // NQ):
                                       (q + 1) * (HID // NQ)] if EP else
                         g["w1"][l][:, e, :, q * (HID // NQ):
                                    (q + 1) * (HID // NQ)]))
                    w2q = wmoe.tile([128, HQ, D], BF16, tag="w2q")
                    nc.sync.dma_start(
                        w2q[:],
                        (g["w2"][l][e][:, q * HQ:(q + 1) * HQ, :] if EP else
                         g["w2"][l][:, e, q * HQ:(q + 1) * HQ, :]))
                    hid = act2.tile([128, HQ, S2 if EP else S], BF16,
                                    tag="hid")
                    for m in range(HQ):
                        ps = mm.tile([128, 512], F32, tag="mm")
                        for kc in range(KC):
                            nc.tensor.matmul(
                                ps[:, :ws], w1q[:, kc, 128 * m:128 * (m + 1)],
                                yb[:, kc, :], start=(kc == 0),
                                stop=(kc == KC - 1))
                        nc.scalar.activation(
                            hid[:, m, :ws], ps[:, :ws], AF.Gelu_apprx_tanh,
                            bias=lnp["b1b"][:, l, e,
                                            q * HQ + m:q * HQ + m + 1],
                            scale=1.0)
                    for m in range(KC):
                        ps = mm.tile([128, 512], F32, tag="mm")
                        for kc in range(HQ):
                            last = (q < NQ - 1 and kc == HQ - 1)
                            nc.tensor.matmul(
                                ps[:, :ws], w2q[:, kc, 128 * m:128 * (m + 1)],
                                hid[:, kc, :ws], start=(kc == 0), stop=last)
                        if q == NQ - 1:
                            # add b2 for every token; drain scale by selgv
                            # zeroes it for unselected tokens
                            nc.tensor.matmul(
                                ps[:, :ws],
                                b2row_sb[:, e, 128 * m:128 * (m + 1)],
                                ones_row_S[:, :ws], start=False, stop=True)
                        tmp = act2.tile([128, S2 if EP else S], F32,
                                        tag="moedr")
                        nc.vector.tensor_tensor(tmp[:, :ws], ps[:, :ws],
                                                pselb, ALU.mult)
                        if first and q == 0:
                            nc.vector.tensor_copy(moeacc[:, m, :],
                                                  tmp[:, :ws])
                        else:
                            nc.vector.tensor_tensor(moeacc[:, m, :],
                                                    moeacc[:, m, :],
                                                    tmp[:, :ws], ALU.add)

            if EP:
                y32 = act.tile([128, KC, S], F32, tag="y32")
                layernorm(l, "ln2g", "ln2b", None, y32)
                ybounce = dram2.tile([128, KC, S], F32, tag="yb",
                                     name="ybounce")
                nc.sync.dma_start(ybounce[:], y32[:])
                ypair = dram2.tile([2, 128, KC, S], F32, tag="yp",
                                   name="ypair")
                nc.gpsimd.collective_compute(
                    "AllGather", ALU.bypass, replica_groups=GPAIR,
                    ins=[ybounce[:].opt()], outs=[ypair[:].opt()])
                postA(l + 2)
                y32p = act.tile([128, KC, S2], F32, tag="y32p")
                for i in range(2):
                    nc.sync.dma_start(y32p[:, :, i * S:(i + 1) * S],
                                      ypair[i])
                y2 = act.tile([128, KC, S2], BF16, tag="y2")
                for kc in range(KC):
                    nc.vector.tensor_copy(y2[:, kc, :], y32p[:, kc, :])
                ones_row_S = act2.tile([1, S2], BF16, tag="onesS")
                nc.vector.memset(ones_row_S[:], 1.0)
                selgv = [[act2.tile([1, S], F32, tag=f"sel{i}{e}",
                                    name=f"sel{i}{e}") for e in range(EH)]
                         for i in range(2)]
                for i in range(2):
                    gate_masks(y32p, i * S, selgv[i])
                moe2 = act.tile([128, KC, S2], F32, tag="moe")
                for e in range(EH):
                    pselbt = mm.tile([128, 512], F32, tag="mm", name="pselb")
                    for i in range(2):
                        nc.tensor.matmul(pselbt[:, i * S:(i + 1) * S],
                                         ones_row_f32[:], selgv[i][e][:],
                                         start=True, stop=True)
                    pselb_sb = act2.tile([128, S2], F32, tag="pselb_sb")
                    nc.vector.tensor_copy(pselb_sb[:], pselbt[:, :S2])
                    expert_ffn(e, y2, S2, pselb_sb[:], moe2, e == 0)
                moebf = act2.tile([128, KC, S2], BF16, tag="hid", name="moebf")
                for kc in range(KC):
                    nc.vector.tensor_copy(moebf[:, kc, :], moe2[:, kc, :])
                moebounce = dram2.tile([2, 128, KC, S], BF16, tag="mb",
                                       name="moebounce")
                for i in range(2):
                    nc.sync.dma_start(moebounce[i],
                                      moebf[:, :, i * S:(i + 1) * S])
                moeout = dram2.tile([128, KC, S], BF16, tag="mo",
                                    name="moeout")
                nc.gpsimd.collective_compute(
                    "ReduceScatter", ALU.add, replica_groups=GPAIR,
                    ins=[moebounce[:].opt()], outs=[moeout[:].opt()])
                postB(l + 2)
                moesb = act2.tile([128, KC, S], BF16, tag="moesb")
                nc.sync.dma_start(moesb[:], moeout[:])
                moes32 = act2.tile([128, KC, S], F32, tag="moes32")
                for kc in range(KC):
                    nc.vector.tensor_copy(moes32[:, kc, :], moesb[:, kc, :])
                for m in range(KC):
                    nc.vector.tensor_tensor(h[:, m, :], h[:, m, :],
                                            moes32[:, m, :], ALU.add)
            else:
                y = act.tile([128, KC, S], BF16, tag="y")
                y32 = act.tile([128, KC, S], F32, tag="y32")
                layernorm(l, "ln2g", "ln2b", y, y32)
                ones_row_S = act2.tile([1, S], BF16, tag="onesS")
                nc.vector.memset(ones_row_S[:], 1.0)
                selgv = [act2.tile([1, S], F32, tag=f"sel{e}",
                                   name=f"sel{e}") for e in range(E)]
                gate_masks(y32, 0, selgv)
                moe = act.tile([128, KC, S], F32, tag="moe")
                for e in range(E):
                    pselbt = mm.tile([128, 512], F32, tag="mm", name="pselb")
                    nc.tensor.matmul(pselbt[:, :S], ones_row_f32[:],
                                     selgv[e][:], start=True, stop=True)
                    pselb_sb = act2.tile([128, S], F32, tag="pselb_sb")
                    nc.vector.tensor_copy(pselb_sb[:], pselbt[:, :S])
                    expert_ffn(e, y, S, pselb_sb[:], moe, e == 0)
                for m in range(KC):
                    nc.vector.tensor_tensor(h[:, m, :], h[:, m, :],
                                            moe[:, m, :], ALU.add)

        # ---- head (cls token only) ----
        head_b_sb = act.tile([1, NC], F32, tag="headb")
        nc.sync.dma_start(head_b_sb[:], t["head_b"][:])
        cls_bf = act.tile([128, KC], BF16, tag="clsbf")
        nc.vector.tensor_copy(cls_bf[:], h[:, :, 0])
        logit = act.tile([1, NC], F32, tag="headout")
        for i in range(4):
            n0, nw = 250 * i, 250
            hw_sb = wmoe.tile([128, KC, nw], BF16, tag="w2q", name="hw_sb")
            nc.sync.dma_start(hw_sb[:], g["hw"][:, :, n0:n0 + nw])
            ps = mm.tile([128, 512], F32, tag="mm")
            for kc in range(KC):
                nc.tensor.matmul(ps[:1, :nw], cls_bf[:, kc:kc + 1],
                                 hw_sb[:, kc, :], start=(kc == 0),
                                 stop=(kc == KC - 1))
            nc.vector.tensor_tensor(logit[:, n0:n0 + nw], ps[:1, :nw],
                                    head_b_sb[:, n0:n0 + nw], ALU.add)
        nc.sync.dma_start(t["out"][:], logit[:])
        if t["hdbg"] is not None:
            nc.sync.dma_start(t["hdbg"][:], h[:])


def _prep(inputs):
    bf = ml_dtypes.bfloat16
    f32 = np.float32
    shared = {}
    big = {}
    # patch embedding weight: [D, CIN, P, P] -> [CIN*P*P, D] -> [128, KC, D]
    pw = inputs["patch_w"].reshape(D, CIN * PATCH * PATCH).T
    big["pwT"] = np.ascontiguousarray(
        pw.reshape(KC, 128, D).transpose(1, 0, 2)).astype(bf)
    shared["patch_b"] = np.ascontiguousarray(
        inputs["patch_b"].reshape(KC, 128).T).astype(f32)
    pos = inputs["pos_embed"][0] + np.concatenate(
        [inputs["cls_token"][0], np.zeros((NPAT, D), f32)], 0)  # [S, D]
    big["posT"] = np.ascontiguousarray(
        pos.T.reshape(KC, 128, S).transpose(1, 0, 2)).astype(f32)

    w = inputs["attn_in_w"]  # [12, 3D, D]
    big["wqkvT"] = np.ascontiguousarray(
        w.transpose(0, 2, 1).reshape(NDEPTH, KC, 128, 3 * D)
        .transpose(0, 2, 1, 3)).astype(bf)  # [12, 128, KC, 3D]
    shared["bqkv"] = np.ascontiguousarray(
        inputs["attn_in_b"].reshape(NDEPTH, 18, 128).transpose(2, 0, 1)
    ).astype(f32)
    shared["bqkv_row"] = inputs["attn_in_b"].reshape(
        1, NDEPTH, 3 * D).astype(bf)
    big["woT"] = np.ascontiguousarray(
        inputs["attn_out_w"].transpose(0, 2, 1).reshape(NDEPTH, KC, 128, D)
        .transpose(0, 2, 1, 3)).astype(bf)  # [12, 128, KC, D]
    shared["bo"] = np.ascontiguousarray(
        inputs["attn_out_b"].reshape(NDEPTH, KC, 128).transpose(2, 0, 1)
    ).astype(f32)
    for src, dst in (("ln1_g", "ln1g"), ("ln1_b", "ln1b"),
                     ("ln2_g", "ln2g"), ("ln2_b", "ln2b")):
        shared[dst] = np.ascontiguousarray(
            inputs[src].reshape(NDEPTH, KC, 128).transpose(2, 0, 1)
        ).astype(f32)
    shared["gwT32"] = np.ascontiguousarray(
        inputs["gate_w"].transpose(0, 2, 1).reshape(NDEPTH, KC, 128, E)
        .transpose(2, 0, 1, 3)).astype(f32)
    shared["gb_row"] = inputs["gate_b"].reshape(1, NDEPTH, E).astype(f32)
    if EP:
        # expert-major per-layer layout so a contiguous half of the flat
        # buffer is exactly two experts (gathered by the 4-core groups)
        big["w1T"] = np.ascontiguousarray(
            inputs["w1"].reshape(NDEPTH, E, KC, 128, HID)
            .transpose(0, 1, 3, 2, 4)).astype(bf)  # [12, E, 128, KC, HID]
        big["w2T"] = np.ascontiguousarray(
            inputs["w2"].reshape(NDEPTH, E, HC, 128, D)
            .transpose(0, 1, 3, 2, 4)).astype(bf)  # [12, E, 128, HC, D]
    else:
        big["w1T"] = np.ascontiguousarray(
            inputs["w1"].reshape(NDEPTH, E, KC, 128, HID)
            .transpose(0, 3, 1, 2, 4)).astype(bf)  # [12, 128, E, KC, HID]
        big["w2T"] = np.ascontiguousarray(
            inputs["w2"].reshape(NDEPTH, E, HC, 128, D)
            .transpose(0, 3, 1, 2, 4)).astype(bf)  # [12, 128, E, HC, D]
    shared["b1b"] = np.ascontiguousarray(
        inputs["b1"].reshape(NDEPTH, E, HC, 128).transpose(3, 0, 1, 2)
    ).astype(f32)
    shared["b2row"] = inputs["b2"].reshape(1, NDEPTH, E, D).astype(bf)
    big["hwT"] = np.ascontiguousarray(
        inputs["head_w"].T.reshape(KC, 128, NC).transpose(1, 0, 2)).astype(bf)
    shared["head_b"] = inputs["head_b"].reshape(1, NC).astype(f32)

    # per-core image patches, feature-major [128, KC, NPAT]
    x = inputs["x"]  # [B, CIN, IMG, IMG]
    xp = x.reshape(B, CIN, 14, PATCH, 14, PATCH).transpose(0, 1, 3, 5, 2, 4)
    xp = xp.reshape(B, CIN * PATCH * PATCH, NPAT)  # [B, 768, 196]
    xps = [np.ascontiguousarray(
        xp[b].reshape(KC, 128, NPAT).transpose(1, 0, 2)).astype(bf)
        for b in range(B)]
    return shared, big, xps


def _shard_big(big, c):
    """Per-core 1/8 contiguous shard of each replicated weight tensor."""
    out = {}
    if GATHER:
        for name, key, per_layer in (("wqkvT", "wqkvT_s", True),
                                     ("woT", "woT_s", True),
                                     ("w1T", "w1T_s", True),
                                     ("w2T", "w2T_s", True),
                                     ("pwT", "pwT_s", False),
                                     ("posT", "posT_s", False),
                                     ("hwT", "hwT_s", False)):
            a = big[name]
            if per_layer:
                flat = a.reshape(NDEPTH, -1)
                n8 = flat.shape[1] // N_CORES
                out[key] = np.ascontiguousarray(
                    flat[:, c * n8:(c + 1) * n8])
            else:
                flat = a.reshape(-1)
                n8 = flat.shape[0] // N_CORES
                out[key] = np.ascontiguousarray(flat[c * n8:(c + 1) * n8])
    else:
        for name in ("wqkvT", "woT", "w1T", "w2T"):
            out[name + "_g"] = big[name]
        for name in ("pwT", "posT", "hwT"):
            out[name + "_g"] = big[name]
    return out


def kernel(**inputs):
    inputs = {k: np.asarray(v) for k, v in inputs.items()}
    shared, big, xps = _prep(inputs)
    nc = _build(DEPTH)
    in_maps = []
    for c in range(N_CORES):
        m = dict(shared, xp=xps[c], **_shard_big(big, c))
        if EP:
            # core-resident expert half: biases sliced, gate columns
            # permuted so this core's experts are always columns 0..EH-1
            half = int(c >= 4)
            perm = [0, 1, 2, 3] if half == 0 else [2, 3, 0, 1]
            m["b1b"] = np.ascontiguousarray(
                shared["b1b"][:, :, 2 * half:2 * half + EH])
            m["b2row"] = np.ascontiguousarray(
                shared["b2row"][:, :, 2 * half:2 * half + EH])
            m["gwT32"] = np.ascontiguousarray(shared["gwT32"][..., perm])
            m["gb_row"] = np.ascontiguousarray(shared["gb_row"][..., perm])
        in_maps.append(m)
    trace = bool(int(os.environ.get("VIT_TRACE", "0")))
    res = run_bass_kernel_spmd(nc, in_maps, list(range(N_CORES)),
                               trace=trace)
    kernel._nc = nc
    kernel._in_maps = in_maps
    kernel._decls = {k: v for k, v in _LAST_DECLS.items() if k != "out"}
    out = np.stack([res.results[c]["out"][0] for c in range(N_CORES)])
    if DEBUG_H:
        kernel._hdbg = [res.results[c]["hdbg"] for c in range(N_CORES)]
    kernel._res = res
    return out.astype(np.float32)
